# revision 8
# baseline (speedup 1.0000x reference)
"""MHNA (masked, exp(n)-normalized multi-head attention) Trainium2 Bass kernel.

Sharding: 8 cores = batch(2) x head-groups(4 heads each). Core c (b=c//4,
g=c%4) receives only 1/8 of x (its 512-token slice of batch b, transposed,
bf16) plus its 4 heads' weight slices (bf16). On device: AllGather x over the
4-core batch group, compute the 4 heads' attention + the wo-row slice of the
output projection (with bo/4 folded in), then ReduceScatter the partial
(2048,1024) output over the batch group so each core returns the final
(512,1024) rows for its token slice. Host just concatenates.

This minimizes bytes shipped through the per-exec dispatch path (which moves
every operand on every call): ~5MB/core in + 2MB/core out instead of the
~20MB/core in + 8MB/core out of the unsharded-x / host-reduced variant.

Device layout choices (validated against a numpy mirror):
  - x arrives pre-transposed (xs = x[b].T columns) so every projection streams
    with the contraction dim (D) on partitions.
  - Q/K are produced transposed (QT/KT = W.T @ xT) in head-pair tiles [128, S]:
    partitions 0:64 = even head, 64:128 = odd head. Scores then run as K=64
    row-packed matmuls (two heads concurrently in the PE array).
  - The causal mask and the exp(n) normalizer: scores*mask/exp(n_t). The
    normalizer is folded into Q (q_t scaled by exp(-n_t) before scores); the
    mask is a single sliding-window Const tile maskB[128, 896] applied during
    the PSUM->SBUF evacuation of diagonal score blocks.
  - ctx is produced transposed (ctxT = V.T @ ST) with col-packed M=64 matmuls
    (two heads concurrently), which feeds the out-projection directly as lhsT.
  - Biases: per-partition biases (bq/bk/bn) via ACT/DVE ops; biases that vary
    along the free dim (bv, bo/4) via K=1 rank-1 matmuls accumulated into the
    same PSUM.
"""
import numpy as np
import ml_dtypes

import concourse.bacc as bacc
import concourse.mybir as mybir
import concourse.tile as tile
from concourse.bass_utils import run_bass_kernel_spmd

F32 = mybir.dt.float32
BF16 = mybir.dt.bfloat16
MMDT = mybir.dt.float32r
AF = mybir.ActivationFunctionType
ALU = mybir.AluOpType
NPBF16 = ml_dtypes.bfloat16

B, S, D, H, DH = 2, 2048, 1024, 16, 64
HL = 4            # heads per core
NTG = 4           # t groups of 512
NTC = 16          # t chunks of 128
SS = S // 4       # 512-token shard per core

# Operands are packed aggressively: the dispatch path charges a per-operand
# cost on every call, so ship few, larger tensors.
#   wx   = [ xs | wq | wk | wv | wn ] along columns, bf16 (D, 512+772)
#   bf32 = [ bq | bk | bnc-col ] f32 (128, 5)
#   bmm  = [ sel | bvr | bod4 | ones ] f32r rows/cols packed (4, 1664)
_IN_SPECS = dict(
    wx=((D, SS + 772), BF16),
    wo=((256, D), BF16),
    bf32=((128, 5), F32),
    bmm=((HL, 1664), MMDT),
)

RG = [[0, 1, 2, 3], [4, 5, 6, 7]]   # batch groups


def _kernel_body(tc, out, ins, mask_dram, phases=(1, 2, 3)):
    nc = tc.nc
    with (
        tc.tile_pool(name="dram", bufs=1, space="DRAM") as dp,
        tc.tile_pool(name="const", bufs=1) as cp,
        tc.tile_pool(name="xtp", bufs=2) as xtp,
        tc.tile_pool(name="big", bufs=1) as bigp,
        tc.tile_pool(name="stp", bufs=6) as stp,
        tc.tile_pool(name="outp", bufs=2) as outp,
        tc.tile_pool(name="ps_st", bufs=3, space="PSUM") as ps_st,
        tc.tile_pool(name="ps_ctx", bufs=2, space="PSUM") as ps_ctx,
        tc.tile_pool(name="ps_gen", bufs=2, space="PSUM") as ps_gen,
        tc.tile_pool(name="ps_vn", bufs=1, space="PSUM") as ps_vn,
    ):
        # ---- x AllGather over the batch group (starts immediately) ----
        xg_in = dp.tile([D, SS], BF16)
        xg_out = dp.tile([4 * D, SS], BF16)
        nc.sync.dma_start(xg_in[:], ins["wx"][:, 0:SS])
        nc.gpsimd.collective_compute(
            "AllGather", ALU.bypass, replica_groups=RG,
            ins=[xg_in.opt()], outs=[xg_out.opt()])

        po = dp.tile([S, D], BF16)                # partial out (pre-reduce)
        ro = dp.tile([SS, D], BF16)

        # ---- constants / weights to SBUF ----
        wcat_sb = cp.tile([128, 8, 772], BF16)
        wo_bf = cp.tile([128, 2, D], BF16)
        wo_sb = cp.tile([128, 2, D], MMDT)
        nc.sync.dma_start(
            wcat_sb[:], ins["wx"][:, SS:].rearrange("(a p) c -> p a c", p=128))
        nc.sync.dma_start(wo_bf[:], ins["wo"].rearrange("(a p) c -> p a c", p=128))
        for p in range(2):
            nc.scalar.copy(wo_sb[:, p, :], wo_bf[:, p, :])
        wq_sb = wcat_sb[:, :, 0:256]
        wk_sb = wcat_sb[:, :, 256:512]
        wv_sb = wcat_sb[:, :, 512:768]
        wn_sb = wcat_sb[:, :, 768:772]
        bf32_sb = cp.tile([128, 5], F32)
        bmm_sb = cp.tile([HL, 1664], MMDT)
        mask_sb = cp.tile([128, 896], F32)
        nc.sync.dma_start(bf32_sb[:], ins["bf32"][:])
        nc.sync.dma_start(bmm_sb[:], ins["bmm"][:])
        nc.sync.dma_start(mask_sb[:], mask_dram[:])
        bq_sb = bf32_sb[:, 0:2]
        bk_sb = bf32_sb[:, 2:4]
        bnc_sb = bf32_sb[0:4, 4:5]
        sel_sb = bmm_sb[:, 0:256]
        bvr_sb = bmm_sb[0:1, 256:512]
        bod4_sb = bmm_sb[0:1, 512:1536]
        ones_sb = bmm_sb[0:1, 1536:1664]

        qt_sb = bigp.tile([128, 2, S], MMDT)      # [part, pair, t]
        kt_sb = bigp.tile([128, 2, S], MMDT)
        v_sb = bigp.tile([128, NTC, 256], MMDT)   # [s-in-chunk, chunk, hc]
        wt_sb = bigp.tile([HL, S], MMDT)          # exp(-(n+bn)) per local head
        ctxt_sb = bigp.tile([128, 2, S], MMDT)    # [pair-dv, pair, t]

        # ================= stage 1: projections =================
        for tg in range(NTG if 1 in phases else 0):
            tsl = slice(tg * 512, (tg + 1) * 512)
            xt_tg = xtp.tile([128, 8, 512], BF16, tag="xt")
            nc.sync.dma_start(
                xt_tg[:],
                xg_out[D * tg:D * (tg + 1), :].rearrange("(a p) t -> p a t", p=128))

            # N-projection -> wT = exp(-(n_pre + bn))
            n_ps = ps_vn.tile([HL, 512], F32, tag="v")
            for dc in range(8):
                nc.tensor.matmul(n_ps[:], wn_sb[:, dc, :], xt_tg[:, dc, :],
                                 start=(dc == 0), stop=(dc == 7))
            nc.scalar.activation(wt_sb[:, tsl], n_ps[:], AF.Exp,
                                 bias=bnc_sb[:], scale=-1.0)

            for pair in range(2):
                psl = slice(128 * pair, 128 * pair + 128)
                # wrep[p, t] = exp(-n) broadcast: partitions 0:64 <- even head
                wrep_ps = ps_gen.tile([128, 512], F32, tag="gen")
                nc.tensor.matmul(wrep_ps[:], sel_sb[:, psl], wt_sb[:, tsl],
                                 start=True, stop=True)
                wrep_sb = outp.tile([128, 512], F32, tag="wrep_sb")
                nc.scalar.copy(wrep_sb[:], wrep_ps[:])
                # QT
                q_ps = ps_gen.tile([128, 512], F32, tag="gen")
                for dc in range(8):
                    nc.tensor.matmul(q_ps[:], wq_sb[:, dc, psl], xt_tg[:, dc, :],
                                     start=(dc == 0), stop=(dc == 7))
                nc.vector.scalar_tensor_tensor(
                    qt_sb[:, pair, tsl], q_ps[:], bq_sb[:, pair:pair + 1],
                    wrep_sb[:], ALU.add, ALU.mult)
                # KT
                k_ps = ps_gen.tile([128, 512], F32, tag="gen")
                for dc in range(8):
                    nc.tensor.matmul(k_ps[:], wk_sb[:, dc, psl], xt_tg[:, dc, :],
                                     start=(dc == 0), stop=(dc == 7))
                nc.scalar.activation(kt_sb[:, pair, tsl], k_ps[:], AF.Identity,
                                     bias=bk_sb[:, pair:pair + 1])

            # V (+bias via rank-1 matmul)
            for tl in range(4):
                tc16 = tg * 4 + tl
                v_ps = ps_vn.tile([128, 256], F32, tag="v")
                for dc in range(8):
                    nc.tensor.matmul(v_ps[:], xt_tg[:, dc, tl * 128:(tl + 1) * 128],
                                     wv_sb[:, dc, :], start=(dc == 0), stop=False)
                nc.tensor.matmul(v_ps[:], ones_sb[:], bvr_sb[:],
                                 start=False, stop=True)
                if tl % 2 == 0:
                    nc.vector.tensor_copy(v_sb[:, tc16, :], v_ps[:])
                else:
                    nc.scalar.copy(v_sb[:, tc16, :], v_ps[:])

        # ================= stage 2+3: scores + ctx =================
        ndve = 0
        for pair in range(2 if 2 in phases else 0):
            for tg in range(NTG):
                tsl = slice(tg * 512, (tg + 1) * 512)
                ctx_ps = [ps_ctx.tile([64, 512], F32, tag="ctx", name=f"ctx{_h}") for _h in range(2)]
                nblk = 4 * tg + 4
                prev_sb, prev_j = None, -1
                for j in range(nblk):
                    st_list = []
                    for hh in range(2):
                        hsl = slice(64 * hh, 64 * hh + 64)
                        st_ps = ps_st.tile([128, 512], F32, tag="st")
                        nc.tensor.matmul(
                            st_ps[:], kt_sb[hsl, pair, j * 128:(j + 1) * 128],
                            qt_sb[hsl, pair, tsl], start=True, stop=True,
                            tile_position=(64 * hh, 0))
                        st_list.append(st_ps)
                    cur_sb = []
                    for hh in range(2):
                        st_sb = stp.tile([128, 512], MMDT, tag="st_sb")
                        r = j - 4 * tg
                        if r >= 0:
                            nc.vector.tensor_mul(
                                st_sb[:], st_list[hh][:],
                                mask_sb[:, 384 - 128 * r: 896 - 128 * r])
                        else:
                            ndve += 1
                            if ndve % 4 == 0:
                                nc.vector.tensor_copy(st_sb[:], st_list[hh][:])
                            else:
                                nc.scalar.copy(st_sb[:], st_list[hh][:])
                        cur_sb.append(st_sb)
                    if prev_sb is not None:
                        for hh in range(2):
                            hl_g = 2 * pair + hh
                            nc.tensor.matmul(
                                ctx_ps[hh][:],
                                v_sb[:, prev_j, 64 * hl_g:64 * hl_g + 64],
                                prev_sb[hh][:],
                                start=(prev_j == 0), stop=False)
                    prev_sb, prev_j = cur_sb, j
                for hh in range(2):
                    hl_g = 2 * pair + hh
                    nc.tensor.matmul(
                        ctx_ps[hh][:],
                        v_sb[:, prev_j, 64 * hl_g:64 * hl_g + 64], prev_sb[hh][:],
                        start=(prev_j == 0), stop=True)
                for hh in range(2):
                    if (tg + hh) % 2 == 0:
                        nc.vector.tensor_copy(ctxt_sb[64*hh:64*hh+64, pair, tsl], ctx_ps[hh][:])
                    else:
                        nc.scalar.copy(ctxt_sb[64*hh:64*hh+64, pair, tsl], ctx_ps[hh][:])

        # ================= stage 4: out projection + reduce =================
        for tc16 in range(NTC if 3 in phases else 0):
            csl = slice(tc16 * 128, (tc16 + 1) * 128)
            out_sb = outp.tile([128, D], BF16, tag="out")
            for eb in range(2):
                esl = slice(eb * 512, (eb + 1) * 512)
                o_ps = ps_gen.tile([128, 512], F32, tag="gen")
                for pair in range(2):
                    nc.tensor.matmul(o_ps[:], ctxt_sb[:, pair, csl],
                                     wo_sb[:, pair, esl],
                                     start=(pair == 0), stop=False)
                nc.tensor.matmul(o_ps[:], ones_sb[:], bod4_sb[:, esl],
                                 start=False, stop=True)
                if eb == 0:
                    nc.vector.tensor_copy(out_sb[:, esl], o_ps[:])
                else:
                    nc.scalar.copy(out_sb[:, esl], o_ps[:])
            nc.sync.dma_start(po[csl, :], out_sb[:])

        if 3 in phases:
            nc.gpsimd.collective_compute(
                "ReduceScatter", ALU.add, replica_groups=RG,
                ins=[po.opt()], outs=[ro.opt()])
            nc.sync.dma_start(out[:], ro[:])


def build_nc(phases=(1, 2, 3)):
    nc = bacc.Bacc("TRN2", target_bir_lowering=False, debug=False, num_devices=8)
    ins = {k: nc.dram_tensor(k, list(s), dt, kind="ExternalInput").ap()
           for k, (s, dt) in _IN_SPECS.items()}
    out = nc.dram_tensor("out", [SS, D], BF16, kind="ExternalOutput").ap()
    mask_dram = nc.inline_tensor(_make_maskB(), name="maskB").ap()
    with tile.TileContext(nc) as tc:
        _kernel_body(tc, out, ins, mask_dram, phases=phases)
    nc.compile()
    return nc


def _make_maskB():
    m = np.zeros((128, 896), dtype=np.float32)
    s = np.arange(128)[:, None]
    c = np.arange(896)[None, :]
    m[(c >= 384) & ((c - 384) >= s)] = 1.0
    m[:, 512:] = 1.0
    return m


def core_inputs(inp, c):
    b, hg = c // 4, c % 4
    heads = list(range(4 * hg, 4 * hg + 4))
    x = np.asarray(inp["x"], dtype=np.float32)
    Wqk = np.asarray(inp["Wqk"], dtype=np.float32)
    bqk = np.asarray(inp["bqk"], dtype=np.float32)
    Wv = np.asarray(inp["Wv"], dtype=np.float32)
    bv = np.asarray(inp["bv"], dtype=np.float32)
    Wn = np.asarray(inp["Wn"], dtype=np.float32)
    bn = np.asarray(inp["bn"], dtype=np.float32)
    Wo = np.asarray(inp["Wo"], dtype=np.float32)
    bo = np.asarray(inp["bo"], dtype=np.float32)
    d = {}
    wx = np.empty((D, SS + 772), dtype=NPBF16)
    wx[:, 0:SS] = x[b].T[:, SS * hg:SS * (hg + 1)]
    wx[:, SS:SS + 256] = np.concatenate(
        [Wqk[:, h * 64:(h + 1) * 64] for h in heads], 1)
    wx[:, SS + 256:SS + 512] = np.concatenate(
        [Wqk[:, 1024 + h * 64:1024 + (h + 1) * 64] for h in heads], 1)
    wx[:, SS + 512:SS + 768] = np.concatenate(
        [Wv[:, h * 64:(h + 1) * 64] for h in heads], 1)
    wx[:, SS + 768:SS + 772] = Wn[:, heads]
    d["wx"] = wx
    d["wo"] = np.concatenate([Wo[h * 64:(h + 1) * 64, :] for h in heads], 0).astype(NPBF16)
    bf32 = np.zeros((128, 5), dtype=np.float32)
    bf32[:, 0:2] = np.concatenate([bqk[h * 64:(h + 1) * 64] for h in heads]).reshape(2, 128).T
    bf32[:, 2:4] = np.concatenate([bqk[1024 + h * 64:1024 + (h + 1) * 64] for h in heads]).reshape(2, 128).T
    bf32[0:4, 4] = -bn[heads]
    d["bf32"] = bf32
    bmm = np.zeros((4, 1664), dtype=np.float32)
    for p in range(2):
        bmm[2 * p + 0, 128 * p:128 * p + 64] = 1.0        # sel
        bmm[2 * p + 1, 128 * p + 64:128 * p + 128] = 1.0
    bmm[0, 256:512] = np.concatenate([bv[h * 64:(h + 1) * 64] for h in heads])
    bmm[0, 512:1536] = bo / 4.0
    bmm[0, 1536:1664] = 1.0
    d["bmm"] = bmm
    return {k: np.ascontiguousarray(v) for k, v in d.items()}


_NC_CACHE = {}


def _get_nc():
    if "nc" not in _NC_CACHE:
        _NC_CACHE["nc"] = build_nc()
    return _NC_CACHE["nc"]


def _run(inputs, **spmd_kwargs):
    nc = _get_nc()
    in_maps = [core_inputs(inputs, c) for c in range(8)]
    # The tunneled device pool occasionally drops an execution (mesh
    # desync / worker hangup); a fresh attempt usually goes through.
    for attempt in range(3):
        try:
            res = run_bass_kernel_spmd(nc, in_maps, list(range(8)), **spmd_kwargs)
            break
        except Exception:
            if attempt == 2:
                raise
            import time
            time.sleep(10)
    out = np.empty((B, S, D), dtype=np.float32)
    for c in range(8):
        b, hg = c // 4, c % 4
        out[b, SS * hg:SS * (hg + 1), :] = np.asarray(
            res.results[c]["out"], dtype=np.float32)
    return out, res


def kernel(**inputs):
    out, _ = _run(inputs)
    return out


# revision 9
# speedup vs baseline: 1.3363x; 1.3363x over previous
"""MHNA (masked, exp(n)-normalized multi-head attention) Trainium2 Bass kernel.

Sharding: 8 cores = batch(2) x head-groups(4 heads each). Core c (b=c//4,
g=c%4) receives only 1/8 of x (its 512-token slice of batch b, transposed,
bf16) plus its 4 heads' weight slices (bf16). On device: AllGather x over the
4-core batch group, compute the 4 heads' attention + the wo-row slice of the
output projection (with bo/4 folded in), then ReduceScatter the partial
(2048,1024) output over the batch group so each core returns the final
(512,1024) rows for its token slice. Host just concatenates.

This minimizes bytes shipped through the per-exec dispatch path (which moves
every operand on every call): ~5MB/core in + 2MB/core out instead of the
~20MB/core in + 8MB/core out of the unsharded-x / host-reduced variant.

Device layout choices (validated against a numpy mirror):
  - x arrives pre-transposed (xs = x[b].T columns) so every projection streams
    with the contraction dim (D) on partitions.
  - Q/K are produced transposed (QT/KT = W.T @ xT) in head-pair tiles [128, S]:
    partitions 0:64 = even head, 64:128 = odd head. Scores then run as K=64
    row-packed matmuls (two heads concurrently in the PE array).
  - The causal mask and the exp(n) normalizer: scores*mask/exp(n_t). The
    normalizer is folded into Q (q_t scaled by exp(-n_t) before scores); the
    mask is a single sliding-window Const tile maskB[128, 896] applied during
    the PSUM->SBUF evacuation of diagonal score blocks.
  - ctx is produced transposed (ctxT = V.T @ ST) with col-packed M=64 matmuls
    (two heads concurrently), which feeds the out-projection directly as lhsT.
  - Biases: per-partition biases (bq/bk/bn) via ACT/DVE ops; biases that vary
    along the free dim (bv, bo/4) via K=1 rank-1 matmuls accumulated into the
    same PSUM.
"""
import numpy as np
import ml_dtypes

import concourse.bacc as bacc
import concourse.mybir as mybir
import concourse.tile as tile
from concourse.bass_utils import run_bass_kernel_spmd

F32 = mybir.dt.float32
BF16 = mybir.dt.bfloat16
MMDT = mybir.dt.float32r
AF = mybir.ActivationFunctionType
ALU = mybir.AluOpType
NPBF16 = ml_dtypes.bfloat16

B, S, D, H, DH = 2, 2048, 1024, 16, 64
HL = 4            # heads per core
NTG = 4           # t groups of 512
NTC = 16          # t chunks of 128
SS = S // 4       # 512-token shard per core

# Operands are packed aggressively: the dispatch path charges a per-operand
# cost on every call, so ship few, larger tensors.
#   wx   = [ xs | wq | wk | wv | wn ] along columns, bf16 (D, 512+772)
#   bf32 = [ bq | bk | bnc-col ] f32 (128, 5)
#   bmm  = [ sel | bvr | bod4 | ones ] f32r rows/cols packed (4, 1664)
_IN_SPECS = dict(
    wx=((D, SS + 772), BF16),
    wo=((256, D), BF16),
    bf32=((128, 5), F32),
    bmm=((HL, 1664), MMDT),
)

RG = [[0, 1, 2, 3], [4, 5, 6, 7]]   # batch groups


def _kernel_body(tc, out, ins, mask_dram, phases=(1, 2, 3)):
    nc = tc.nc
    with (
        tc.tile_pool(name="dram", bufs=1, space="DRAM") as dp,
        tc.tile_pool(name="const", bufs=1) as cp,
        tc.tile_pool(name="xtp", bufs=2) as xtp,
        tc.tile_pool(name="big", bufs=1) as bigp,
        tc.tile_pool(name="stp", bufs=6) as stp,
        tc.tile_pool(name="outp", bufs=2) as outp,
        tc.tile_pool(name="ps_st", bufs=3, space="PSUM") as ps_st,
        tc.tile_pool(name="ps_ctx", bufs=2, space="PSUM") as ps_ctx,
        tc.tile_pool(name="ps_gen", bufs=2, space="PSUM") as ps_gen,
        tc.tile_pool(name="ps_vn", bufs=1, space="PSUM") as ps_vn,
    ):
        # ---- x AllGather over the batch group (starts immediately) ----
        xg_in = dp.tile([D, SS], BF16)
        xg_out = dp.tile([4 * D, SS], BF16)
        nc.sync.dma_start(xg_in[:], ins["wx"][:, 0:SS])
        nc.gpsimd.collective_compute(
            "AllGather", ALU.bypass, replica_groups=RG,
            ins=[xg_in.opt()], outs=[xg_out.opt()])

        po = dp.tile([S, D], BF16)                # partial out (pre-reduce)
        ro = dp.tile([SS, D], BF16)

        # ---- constants / weights to SBUF ----
        wcat_sb = cp.tile([128, 8, 772], BF16)
        wo_bf = cp.tile([128, 2, D], BF16)
        wo_sb = cp.tile([128, 2, D], MMDT)
        nc.sync.dma_start(
            wcat_sb[:], ins["wx"][:, SS:].rearrange("(a p) c -> p a c", p=128))
        nc.sync.dma_start(wo_bf[:], ins["wo"].rearrange("(a p) c -> p a c", p=128))
        for p in range(2):
            nc.scalar.copy(wo_sb[:, p, :], wo_bf[:, p, :])
        wq_sb = wcat_sb[:, :, 0:256]
        wk_sb = wcat_sb[:, :, 256:512]
        wv_sb = wcat_sb[:, :, 512:768]
        wn_sb = wcat_sb[:, :, 768:772]
        bf32_sb = cp.tile([128, 5], F32)
        bmm_sb = cp.tile([HL, 1664], MMDT)
        mask_sb = cp.tile([128, 896], F32)
        nc.sync.dma_start(bf32_sb[:], ins["bf32"][:])
        nc.sync.dma_start(bmm_sb[:], ins["bmm"][:])
        nc.sync.dma_start(mask_sb[:], mask_dram[:])
        bq_sb = bf32_sb[:, 0:2]
        bk_sb = bf32_sb[:, 2:4]
        bnc_sb = bf32_sb[0:4, 4:5]
        sel_sb = bmm_sb[:, 0:256]
        bvr_sb = bmm_sb[0:1, 256:512]
        bod4_sb = bmm_sb[0:1, 512:1536]
        ones_sb = bmm_sb[0:1, 1536:1664]

        qt_sb = bigp.tile([128, 2, S], MMDT)      # [part, pair, t]
        kt_sb = bigp.tile([128, 2, S], MMDT)
        v_sb = bigp.tile([128, NTC, 256], MMDT)   # [s-in-chunk, chunk, hc]
        wt_sb = bigp.tile([HL, S], MMDT)          # exp(-(n+bn)) per local head
        ctxt_sb = bigp.tile([128, 2, S], MMDT)    # [pair-dv, pair, t]

        # ================= stage 1: projections =================
        for tg in range(NTG if 1 in phases else 0):
            tsl = slice(tg * 512, (tg + 1) * 512)
            xt_tg = xtp.tile([128, 8, 512], BF16, tag="xt")
            nc.sync.dma_start(
                xt_tg[:],
                xg_out[D * tg:D * (tg + 1), :].rearrange("(a p) t -> p a t", p=128))

            # N-projection -> wT = exp(-(n_pre + bn))
            n_ps = ps_vn.tile([HL, 512], F32, tag="v")
            for dc in range(8):
                nc.tensor.matmul(n_ps[:], wn_sb[:, dc, :], xt_tg[:, dc, :],
                                 start=(dc == 0), stop=(dc == 7))
            nc.scalar.activation(wt_sb[:, tsl], n_ps[:], AF.Exp,
                                 bias=bnc_sb[:], scale=-1.0)

            for pair in range(2):
                psl = slice(128 * pair, 128 * pair + 128)
                # wrep[p, t] = exp(-n) broadcast: partitions 0:64 <- even head
                wrep_ps = ps_gen.tile([128, 512], F32, tag="gen")
                nc.tensor.matmul(wrep_ps[:], sel_sb[:, psl], wt_sb[:, tsl],
                                 start=True, stop=True)
                wrep_sb = outp.tile([128, 512], F32, tag="wrep_sb")
                nc.scalar.copy(wrep_sb[:], wrep_ps[:])
                # QT
                q_ps = ps_gen.tile([128, 512], F32, tag="gen")
                for dc in range(8):
                    nc.tensor.matmul(q_ps[:], wq_sb[:, dc, psl], xt_tg[:, dc, :],
                                     start=(dc == 0), stop=(dc == 7))
                nc.vector.scalar_tensor_tensor(
                    qt_sb[:, pair, tsl], q_ps[:], bq_sb[:, pair:pair + 1],
                    wrep_sb[:], ALU.add, ALU.mult)
                # KT
                k_ps = ps_gen.tile([128, 512], F32, tag="gen")
                for dc in range(8):
                    nc.tensor.matmul(k_ps[:], wk_sb[:, dc, psl], xt_tg[:, dc, :],
                                     start=(dc == 0), stop=(dc == 7))
                nc.scalar.activation(kt_sb[:, pair, tsl], k_ps[:], AF.Identity,
                                     bias=bk_sb[:, pair:pair + 1])

            # V (+bias via rank-1 matmul)
            for tl in range(4):
                tc16 = tg * 4 + tl
                v_ps = ps_vn.tile([128, 256], F32, tag="v")
                for dc in range(8):
                    nc.tensor.matmul(v_ps[:], xt_tg[:, dc, tl * 128:(tl + 1) * 128],
                                     wv_sb[:, dc, :], start=(dc == 0), stop=False)
                nc.tensor.matmul(v_ps[:], ones_sb[:], bvr_sb[:],
                                 start=False, stop=True)
                if tl % 2 == 0:
                    nc.vector.tensor_copy(v_sb[:, tc16, :], v_ps[:])
                else:
                    nc.scalar.copy(v_sb[:, tc16, :], v_ps[:])

        # ================= stage 2+3: scores + ctx =================
        ndve = 0
        for pair in range(2 if 2 in phases else 0):
            for tg in range(NTG):
                tsl = slice(tg * 512, (tg + 1) * 512)
                ctx_ps = [ps_ctx.tile([64, 512], F32, tag="ctx", name=f"ctx{_h}") for _h in range(2)]
                nblk = 4 * tg + 4
                prev_sb, prev_j = None, -1
                for j in range(nblk):
                    st_list = []
                    for hh in range(2):
                        hsl = slice(64 * hh, 64 * hh + 64)
                        st_ps = ps_st.tile([128, 512], F32, tag="st")
                        nc.tensor.matmul(
                            st_ps[:], kt_sb[hsl, pair, j * 128:(j + 1) * 128],
                            qt_sb[hsl, pair, tsl], start=True, stop=True,
                            tile_position=(64 * hh, 0))
                        st_list.append(st_ps)
                    cur_sb = []
                    for hh in range(2):
                        st_sb = stp.tile([128, 512], MMDT, tag="st_sb")
                        r = j - 4 * tg
                        if r >= 0:
                            nc.vector.tensor_mul(
                                st_sb[:], st_list[hh][:],
                                mask_sb[:, 384 - 128 * r: 896 - 128 * r])
                        else:
                            ndve += 1
                            if ndve % 4 == 0:
                                nc.vector.tensor_copy(st_sb[:], st_list[hh][:])
                            else:
                                nc.scalar.copy(st_sb[:], st_list[hh][:])
                        cur_sb.append(st_sb)
                    if prev_sb is not None:
                        for hh in range(2):
                            hl_g = 2 * pair + hh
                            nc.tensor.matmul(
                                ctx_ps[hh][:],
                                v_sb[:, prev_j, 64 * hl_g:64 * hl_g + 64],
                                prev_sb[hh][:],
                                start=(prev_j == 0), stop=False)
                    prev_sb, prev_j = cur_sb, j
                for hh in range(2):
                    hl_g = 2 * pair + hh
                    nc.tensor.matmul(
                        ctx_ps[hh][:],
                        v_sb[:, prev_j, 64 * hl_g:64 * hl_g + 64], prev_sb[hh][:],
                        start=(prev_j == 0), stop=True)
                for hh in range(2):
                    if (tg + hh) % 2 == 0:
                        nc.vector.tensor_copy(ctxt_sb[64*hh:64*hh+64, pair, tsl], ctx_ps[hh][:])
                    else:
                        nc.scalar.copy(ctxt_sb[64*hh:64*hh+64, pair, tsl], ctx_ps[hh][:])

        # ================= stage 4: out projection + reduce =================
        for tc16 in range(NTC if 3 in phases else 0):
            csl = slice(tc16 * 128, (tc16 + 1) * 128)
            out_sb = outp.tile([128, D], BF16, tag="out")
            for eb in range(2):
                esl = slice(eb * 512, (eb + 1) * 512)
                o_ps = ps_gen.tile([128, 512], F32, tag="gen")
                for pair in range(2):
                    nc.tensor.matmul(o_ps[:], ctxt_sb[:, pair, csl],
                                     wo_sb[:, pair, esl],
                                     start=(pair == 0), stop=False)
                nc.tensor.matmul(o_ps[:], ones_sb[:], bod4_sb[:, esl],
                                 start=False, stop=True)
                if eb == 0:
                    nc.vector.tensor_copy(out_sb[:, esl], o_ps[:])
                else:
                    nc.scalar.copy(out_sb[:, esl], o_ps[:])
            nc.sync.dma_start(po[csl, :], out_sb[:])

        if 3 in phases:
            nc.gpsimd.collective_compute(
                "ReduceScatter", ALU.add, replica_groups=RG,
                ins=[po.opt()], outs=[ro.opt()])
            nc.sync.dma_start(out[:], ro[:])


def build_nc(phases=(1, 2, 3)):
    nc = bacc.Bacc("TRN2", target_bir_lowering=False, debug=False, num_devices=8,
                   enable_partition_id=False)
    ins = {k: nc.dram_tensor(k, list(s), dt, kind="ExternalInput").ap()
           for k, (s, dt) in _IN_SPECS.items()}
    out = nc.dram_tensor("out", [SS, D], BF16, kind="ExternalOutput").ap()
    mask_dram = nc.inline_tensor(_make_maskB(), name="maskB").ap()
    with tile.TileContext(nc) as tc:
        _kernel_body(tc, out, ins, mask_dram, phases=phases)
    nc.compile()
    return nc


def _make_maskB():
    m = np.zeros((128, 896), dtype=np.float32)
    s = np.arange(128)[:, None]
    c = np.arange(896)[None, :]
    m[(c >= 384) & ((c - 384) >= s)] = 1.0
    m[:, 512:] = 1.0
    return m


def core_inputs(inp, c):
    b, hg = c // 4, c % 4
    heads = list(range(4 * hg, 4 * hg + 4))
    x = np.asarray(inp["x"], dtype=np.float32)
    Wqk = np.asarray(inp["Wqk"], dtype=np.float32)
    bqk = np.asarray(inp["bqk"], dtype=np.float32)
    Wv = np.asarray(inp["Wv"], dtype=np.float32)
    bv = np.asarray(inp["bv"], dtype=np.float32)
    Wn = np.asarray(inp["Wn"], dtype=np.float32)
    bn = np.asarray(inp["bn"], dtype=np.float32)
    Wo = np.asarray(inp["Wo"], dtype=np.float32)
    bo = np.asarray(inp["bo"], dtype=np.float32)
    d = {}
    wx = np.empty((D, SS + 772), dtype=NPBF16)
    wx[:, 0:SS] = x[b].T[:, SS * hg:SS * (hg + 1)]
    wx[:, SS:SS + 256] = np.concatenate(
        [Wqk[:, h * 64:(h + 1) * 64] for h in heads], 1)
    wx[:, SS + 256:SS + 512] = np.concatenate(
        [Wqk[:, 1024 + h * 64:1024 + (h + 1) * 64] for h in heads], 1)
    wx[:, SS + 512:SS + 768] = np.concatenate(
        [Wv[:, h * 64:(h + 1) * 64] for h in heads], 1)
    wx[:, SS + 768:SS + 772] = Wn[:, heads]
    d["wx"] = wx
    d["wo"] = np.concatenate([Wo[h * 64:(h + 1) * 64, :] for h in heads], 0).astype(NPBF16)
    bf32 = np.zeros((128, 5), dtype=np.float32)
    bf32[:, 0:2] = np.concatenate([bqk[h * 64:(h + 1) * 64] for h in heads]).reshape(2, 128).T
    bf32[:, 2:4] = np.concatenate([bqk[1024 + h * 64:1024 + (h + 1) * 64] for h in heads]).reshape(2, 128).T
    bf32[0:4, 4] = -bn[heads]
    d["bf32"] = bf32
    bmm = np.zeros((4, 1664), dtype=np.float32)
    for p in range(2):
        bmm[2 * p + 0, 128 * p:128 * p + 64] = 1.0        # sel
        bmm[2 * p + 1, 128 * p + 64:128 * p + 128] = 1.0
    bmm[0, 256:512] = np.concatenate([bv[h * 64:(h + 1) * 64] for h in heads])
    bmm[0, 512:1536] = bo / 4.0
    bmm[0, 1536:1664] = 1.0
    d["bmm"] = bmm
    return {k: np.ascontiguousarray(v) for k, v in d.items()}


_NC_CACHE = {}


def _get_nc():
    if "nc" not in _NC_CACHE:
        _NC_CACHE["nc"] = build_nc()
    return _NC_CACHE["nc"]


def _run(inputs, **spmd_kwargs):
    nc = _get_nc()
    in_maps = [core_inputs(inputs, c) for c in range(8)]
    # The tunneled device pool occasionally drops an execution (mesh
    # desync / worker hangup); a fresh attempt usually goes through.
    for attempt in range(3):
        try:
            res = run_bass_kernel_spmd(nc, in_maps, list(range(8)), **spmd_kwargs)
            break
        except Exception:
            if attempt == 2:
                raise
            import time
            time.sleep(10)
    out = np.empty((B, S, D), dtype=np.float32)
    for c in range(8):
        b, hg = c // 4, c % 4
        out[b, SS * hg:SS * (hg + 1), :] = np.asarray(
            res.results[c]["out"], dtype=np.float32)
    return out, res


def kernel(**inputs):
    out, _ = _run(inputs)
    return out


# revision 10
# speedup vs baseline: 1.3791x; 1.0321x over previous
"""MHNA (masked, exp(n)-normalized multi-head attention) Trainium2 Bass kernel.

Sharding: 8 cores = batch(2) x head-groups(4 heads each). Core c (b=c//4,
g=c%4) receives only 1/8 of x (its 512-token slice of batch b, transposed,
bf16) plus its 4 heads' weight slices (bf16). On device: AllGather x over the
4-core batch group, compute the 4 heads' attention + the wo-row slice of the
output projection (with bo/4 folded in), then ReduceScatter the partial
(2048,1024) output over the batch group so each core returns the final
(512,1024) rows for its token slice. Host just concatenates.

This minimizes bytes shipped through the per-exec dispatch path (which moves
every operand on every call): ~5MB/core in + 2MB/core out instead of the
~20MB/core in + 8MB/core out of the unsharded-x / host-reduced variant.

Device layout choices (validated against a numpy mirror):
  - x arrives pre-transposed (xs = x[b].T columns) so every projection streams
    with the contraction dim (D) on partitions.
  - Q/K are produced transposed (QT/KT = W.T @ xT) in head-pair tiles [128, S]:
    partitions 0:64 = even head, 64:128 = odd head. Scores then run as K=64
    row-packed matmuls (two heads concurrently in the PE array).
  - The causal mask and the exp(n) normalizer: scores*mask/exp(n_t). The
    normalizer is folded into Q (q_t scaled by exp(-n_t) before scores); the
    mask is a single sliding-window Const tile maskB[128, 896] applied during
    the PSUM->SBUF evacuation of diagonal score blocks.
  - ctx is produced transposed (ctxT = V.T @ ST) with col-packed M=64 matmuls
    (two heads concurrently), which feeds the out-projection directly as lhsT.
  - Biases: per-partition biases (bq/bk/bn) via ACT/DVE ops; biases that vary
    along the free dim (bv, bo/4) via K=1 rank-1 matmuls accumulated into the
    same PSUM.
"""
import numpy as np
import ml_dtypes

import concourse.bacc as bacc
import concourse.mybir as mybir
import concourse.tile as tile
from concourse.bass_utils import run_bass_kernel_spmd

F32 = mybir.dt.float32
BF16 = mybir.dt.bfloat16
MMDT = mybir.dt.float32r
AF = mybir.ActivationFunctionType
ALU = mybir.AluOpType
NPBF16 = ml_dtypes.bfloat16

B, S, D, H, DH = 2, 2048, 1024, 16, 64
HL = 4            # heads per core
NTG = 4           # t groups of 512
NTC = 16          # t chunks of 128
SS = S // 4       # 512-token shard per core

# Operands are packed aggressively: the dispatch path charges a per-operand
# cost on every call, so ship few, larger tensors.
#   wx  = [ xs | wq | wk | wv | wn ] along columns, bf16 (D, 512+772)
#   bmm = [ sel | bvr | bod4 | ones512 | bq | bk | bn ] f32r packed (4, 2564);
#         all biases are applied as K=1 rank-1 matmuls into the PSUM
#         accumulations, so no separate f32 bias tensor is needed.
_IN_SPECS = dict(
    wx=((D, SS + 772), BF16),
    wo=((256, D), BF16),
    bmm=((HL, 2564), MMDT),
)

RG = [[0, 1, 2, 3], [4, 5, 6, 7]]   # batch groups


def _kernel_body(tc, out, ins, mask_dram, phases=(1, 2, 3)):
    nc = tc.nc
    with (
        tc.tile_pool(name="dram", bufs=1, space="DRAM") as dp,
        tc.tile_pool(name="const", bufs=1) as cp,
        tc.tile_pool(name="xtp", bufs=2) as xtp,
        tc.tile_pool(name="big", bufs=1) as bigp,
        tc.tile_pool(name="stp", bufs=6) as stp,
        tc.tile_pool(name="outp", bufs=2) as outp,
        tc.tile_pool(name="ps_st", bufs=3, space="PSUM") as ps_st,
        tc.tile_pool(name="ps_ctx", bufs=2, space="PSUM") as ps_ctx,
        tc.tile_pool(name="ps_gen", bufs=2, space="PSUM") as ps_gen,
        tc.tile_pool(name="ps_vn", bufs=1, space="PSUM") as ps_vn,
    ):
        # ---- x AllGather over the batch group (starts immediately) ----
        xg_in = dp.tile([D, SS], BF16)
        xg_out = dp.tile([4 * D, SS], BF16)
        nc.sync.dma_start(xg_in[:], ins["wx"][:, 0:SS])
        nc.gpsimd.collective_compute(
            "AllGather", ALU.bypass, replica_groups=RG,
            ins=[xg_in.opt()], outs=[xg_out.opt()])

        po = dp.tile([S, D], BF16)                # partial out (pre-reduce)
        ro = dp.tile([SS, D], BF16)

        # ---- constants / weights to SBUF ----
        wcat_sb = cp.tile([128, 8, 772], BF16)
        wo_bf = cp.tile([128, 2, D], BF16)
        wo_sb = cp.tile([128, 2, D], MMDT)
        nc.sync.dma_start(
            wcat_sb[:], ins["wx"][:, SS:].rearrange("(a p) c -> p a c", p=128))
        nc.sync.dma_start(wo_bf[:], ins["wo"].rearrange("(a p) c -> p a c", p=128))
        for p in range(2):
            nc.scalar.copy(wo_sb[:, p, :], wo_bf[:, p, :])
        wq_sb = wcat_sb[:, :, 0:256]
        wk_sb = wcat_sb[:, :, 256:512]
        wv_sb = wcat_sb[:, :, 512:768]
        wn_sb = wcat_sb[:, :, 768:772]
        bmm_sb = cp.tile([HL, 2564], MMDT)
        mask_sb = cp.tile([128, 896], F32)
        nc.sync.dma_start(bmm_sb[:], ins["bmm"][:])
        nc.sync.dma_start(mask_sb[:], mask_dram[:])
        sel_sb = bmm_sb[:, 0:256]
        bvr_sb = bmm_sb[0:1, 256:512]
        bod4_sb = bmm_sb[0:1, 512:1536]
        ones512_sb = bmm_sb[0:1, 1536:2048]
        ones_sb = bmm_sb[0:1, 1536:1664]
        bq_row = [bmm_sb[0:1, 2048 + 128 * p:2176 + 128 * p] for p in range(2)]
        bk_row = [bmm_sb[0:1, 2304 + 128 * p:2432 + 128 * p] for p in range(2)]
        bn_row = bmm_sb[0:1, 2560:2564]

        qt_sb = bigp.tile([128, 2, S], MMDT)      # [part, pair, t]
        kt_sb = bigp.tile([128, 2, S], MMDT)
        v_sb = bigp.tile([128, NTC, 256], MMDT)   # [s-in-chunk, chunk, hc]
        wt_sb = bigp.tile([HL, S], MMDT)          # exp(-(n+bn)) per local head
        ctxt_sb = bigp.tile([128, 2, S], MMDT)    # [pair-dv, pair, t]

        # ================= stage 1: projections =================
        for tg in range(NTG if 1 in phases else 0):
            tsl = slice(tg * 512, (tg + 1) * 512)
            xt_tg = xtp.tile([128, 8, 512], BF16, tag="xt")
            nc.sync.dma_start(
                xt_tg[:],
                xg_out[D * tg:D * (tg + 1), :].rearrange("(a p) t -> p a t", p=128))

            # N-projection -> wT = exp(-(n_pre + bn))
            n_ps = ps_vn.tile([HL, 512], F32, tag="v")
            for dc in range(8):
                nc.tensor.matmul(n_ps[:], wn_sb[:, dc, :], xt_tg[:, dc, :],
                                 start=(dc == 0), stop=False)
            nc.tensor.matmul(n_ps[:], bn_row, ones512_sb,
                             start=False, stop=True)
            nc.scalar.activation(wt_sb[:, tsl], n_ps[:], AF.Exp, scale=-1.0)

            for pair in range(2):
                psl = slice(128 * pair, 128 * pair + 128)
                # wrep[p, t] = exp(-n) broadcast: partitions 0:64 <- even head
                wrep_ps = ps_gen.tile([128, 512], F32, tag="gen")
                nc.tensor.matmul(wrep_ps[:], sel_sb[:, psl], wt_sb[:, tsl],
                                 start=True, stop=True)
                wrep_sb = outp.tile([128, 512], F32, tag="wrep_sb")
                nc.scalar.copy(wrep_sb[:], wrep_ps[:])
                # QT
                q_ps = ps_gen.tile([128, 512], F32, tag="gen")
                for dc in range(8):
                    nc.tensor.matmul(q_ps[:], wq_sb[:, dc, psl], xt_tg[:, dc, :],
                                     start=(dc == 0), stop=False)
                nc.tensor.matmul(q_ps[:], bq_row[pair], ones512_sb,
                                 start=False, stop=True)
                nc.vector.tensor_mul(qt_sb[:, pair, tsl], q_ps[:], wrep_sb[:])
                # KT
                k_ps = ps_gen.tile([128, 512], F32, tag="gen")
                for dc in range(8):
                    nc.tensor.matmul(k_ps[:], wk_sb[:, dc, psl], xt_tg[:, dc, :],
                                     start=(dc == 0), stop=False)
                nc.tensor.matmul(k_ps[:], bk_row[pair], ones512_sb,
                                 start=False, stop=True)
                nc.scalar.copy(kt_sb[:, pair, tsl], k_ps[:])

            # V (+bias via rank-1 matmul)
            for tl in range(4):
                tc16 = tg * 4 + tl
                v_ps = ps_vn.tile([128, 256], F32, tag="v")
                for dc in range(8):
                    nc.tensor.matmul(v_ps[:], xt_tg[:, dc, tl * 128:(tl + 1) * 128],
                                     wv_sb[:, dc, :], start=(dc == 0), stop=False)
                nc.tensor.matmul(v_ps[:], ones_sb[:], bvr_sb[:],
                                 start=False, stop=True)
                if tl % 2 == 0:
                    nc.vector.tensor_copy(v_sb[:, tc16, :], v_ps[:])
                else:
                    nc.scalar.copy(v_sb[:, tc16, :], v_ps[:])

        # ================= stage 2+3: scores + ctx =================
        ndve = 0
        for pair in range(2 if 2 in phases else 0):
            for tg in range(NTG):
                tsl = slice(tg * 512, (tg + 1) * 512)
                ctx_ps = [ps_ctx.tile([64, 512], F32, tag="ctx", name=f"ctx{_h}") for _h in range(2)]
                nblk = 4 * tg + 4
                prev_sb, prev_j = None, -1
                for j in range(nblk):
                    st_list = []
                    for hh in range(2):
                        hsl = slice(64 * hh, 64 * hh + 64)
                        st_ps = ps_st.tile([128, 512], F32, tag="st")
                        nc.tensor.matmul(
                            st_ps[:], kt_sb[hsl, pair, j * 128:(j + 1) * 128],
                            qt_sb[hsl, pair, tsl], start=True, stop=True,
                            tile_position=(64 * hh, 0))
                        st_list.append(st_ps)
                    cur_sb = []
                    for hh in range(2):
                        st_sb = stp.tile([128, 512], MMDT, tag="st_sb")
                        r = j - 4 * tg
                        if r >= 0:
                            nc.vector.tensor_mul(
                                st_sb[:], st_list[hh][:],
                                mask_sb[:, 384 - 128 * r: 896 - 128 * r])
                        else:
                            ndve += 1
                            if ndve % 4 == 0:
                                nc.vector.tensor_copy(st_sb[:], st_list[hh][:])
                            else:
                                nc.scalar.copy(st_sb[:], st_list[hh][:])
                        cur_sb.append(st_sb)
                    if prev_sb is not None:
                        for hh in range(2):
                            hl_g = 2 * pair + hh
                            nc.tensor.matmul(
                                ctx_ps[hh][:],
                                v_sb[:, prev_j, 64 * hl_g:64 * hl_g + 64],
                                prev_sb[hh][:],
                                start=(prev_j == 0), stop=False)
                    prev_sb, prev_j = cur_sb, j
                for hh in range(2):
                    hl_g = 2 * pair + hh
                    nc.tensor.matmul(
                        ctx_ps[hh][:],
                        v_sb[:, prev_j, 64 * hl_g:64 * hl_g + 64], prev_sb[hh][:],
                        start=(prev_j == 0), stop=True)
                for hh in range(2):
                    if (tg + hh) % 2 == 0:
                        nc.vector.tensor_copy(ctxt_sb[64*hh:64*hh+64, pair, tsl], ctx_ps[hh][:])
                    else:
                        nc.scalar.copy(ctxt_sb[64*hh:64*hh+64, pair, tsl], ctx_ps[hh][:])

        # ================= stage 4: out projection + reduce =================
        for tc16 in range(NTC if 3 in phases else 0):
            csl = slice(tc16 * 128, (tc16 + 1) * 128)
            out_sb = outp.tile([128, D], BF16, tag="out")
            for eb in range(2):
                esl = slice(eb * 512, (eb + 1) * 512)
                o_ps = ps_gen.tile([128, 512], F32, tag="gen")
                for pair in range(2):
                    nc.tensor.matmul(o_ps[:], ctxt_sb[:, pair, csl],
                                     wo_sb[:, pair, esl],
                                     start=(pair == 0), stop=False)
                nc.tensor.matmul(o_ps[:], ones_sb[:], bod4_sb[:, esl],
                                 start=False, stop=True)
                if eb == 0:
                    nc.vector.tensor_copy(out_sb[:, esl], o_ps[:])
                else:
                    nc.scalar.copy(out_sb[:, esl], o_ps[:])
            nc.sync.dma_start(po[csl, :], out_sb[:])

        if 3 in phases:
            nc.gpsimd.collective_compute(
                "ReduceScatter", ALU.add, replica_groups=RG,
                ins=[po.opt()], outs=[ro.opt()])
            nc.sync.dma_start(out[:], ro[:])


def build_nc(phases=(1, 2, 3)):
    nc = bacc.Bacc("TRN2", target_bir_lowering=False, debug=False, num_devices=8,
                   enable_partition_id=False)
    ins = {k: nc.dram_tensor(k, list(s), dt, kind="ExternalInput").ap()
           for k, (s, dt) in _IN_SPECS.items()}
    out = nc.dram_tensor("out", [SS, D], BF16, kind="ExternalOutput").ap()
    mask_dram = nc.inline_tensor(_make_maskB(), name="maskB").ap()
    with tile.TileContext(nc) as tc:
        _kernel_body(tc, out, ins, mask_dram, phases=phases)
    nc.compile()
    return nc


def _make_maskB():
    m = np.zeros((128, 896), dtype=np.float32)
    s = np.arange(128)[:, None]
    c = np.arange(896)[None, :]
    m[(c >= 384) & ((c - 384) >= s)] = 1.0
    m[:, 512:] = 1.0
    return m


def core_inputs(inp, c):
    b, hg = c // 4, c % 4
    heads = list(range(4 * hg, 4 * hg + 4))
    x = np.asarray(inp["x"], dtype=np.float32)
    Wqk = np.asarray(inp["Wqk"], dtype=np.float32)
    bqk = np.asarray(inp["bqk"], dtype=np.float32)
    Wv = np.asarray(inp["Wv"], dtype=np.float32)
    bv = np.asarray(inp["bv"], dtype=np.float32)
    Wn = np.asarray(inp["Wn"], dtype=np.float32)
    bn = np.asarray(inp["bn"], dtype=np.float32)
    Wo = np.asarray(inp["Wo"], dtype=np.float32)
    bo = np.asarray(inp["bo"], dtype=np.float32)
    d = {}
    wx = np.empty((D, SS + 772), dtype=NPBF16)
    wx[:, 0:SS] = x[b].T[:, SS * hg:SS * (hg + 1)]
    wx[:, SS:SS + 256] = np.concatenate(
        [Wqk[:, h * 64:(h + 1) * 64] for h in heads], 1)
    wx[:, SS + 256:SS + 512] = np.concatenate(
        [Wqk[:, 1024 + h * 64:1024 + (h + 1) * 64] for h in heads], 1)
    wx[:, SS + 512:SS + 768] = np.concatenate(
        [Wv[:, h * 64:(h + 1) * 64] for h in heads], 1)
    wx[:, SS + 768:SS + 772] = Wn[:, heads]
    d["wx"] = wx
    d["wo"] = np.concatenate([Wo[h * 64:(h + 1) * 64, :] for h in heads], 0).astype(NPBF16)
    bmm = np.zeros((4, 2564), dtype=np.float32)
    for p in range(2):
        bmm[2 * p + 0, 128 * p:128 * p + 64] = 1.0        # sel
        bmm[2 * p + 1, 128 * p + 64:128 * p + 128] = 1.0
    bmm[0, 256:512] = np.concatenate([bv[h * 64:(h + 1) * 64] for h in heads])
    bmm[0, 512:1536] = bo / 4.0
    bmm[0, 1536:2048] = 1.0                               # ones512
    bq_all = np.concatenate([bqk[h * 64:(h + 1) * 64] for h in heads])
    bk_all = np.concatenate([bqk[1024 + h * 64:1024 + (h + 1) * 64] for h in heads])
    bmm[0, 2048:2304] = bq_all
    bmm[0, 2304:2560] = bk_all
    bmm[0, 2560:2564] = bn[heads]
    d["bmm"] = bmm
    return {k: np.ascontiguousarray(v) for k, v in d.items()}


_NC_CACHE = {}


def _get_nc():
    if "nc" not in _NC_CACHE:
        _NC_CACHE["nc"] = build_nc()
    return _NC_CACHE["nc"]


def _run(inputs, **spmd_kwargs):
    nc = _get_nc()
    in_maps = [core_inputs(inputs, c) for c in range(8)]
    # The tunneled device pool occasionally drops an execution (mesh
    # desync / worker hangup); a fresh attempt usually goes through.
    for attempt in range(3):
        try:
            res = run_bass_kernel_spmd(nc, in_maps, list(range(8)), **spmd_kwargs)
            break
        except Exception:
            if attempt == 2:
                raise
            import time
            time.sleep(10)
    out = np.empty((B, S, D), dtype=np.float32)
    for c in range(8):
        b, hg = c // 4, c % 4
        out[b, SS * hg:SS * (hg + 1), :] = np.asarray(
            res.results[c]["out"], dtype=np.float32)
    return out, res


def kernel(**inputs):
    out, _ = _run(inputs)
    return out
